# revision 5
# baseline (speedup 1.0000x reference)
"""Bass/Trainium2 kernel for nn_LMModel_LSTM: 2-layer LSTM LM, T=128 B=32 H=1024 V=32000.

Sharding (8 cores):
- Gate/H dim sharded: core r owns H-slice [128r, 128r+128) for all 4 gates.
- Embedding: each core gathers its 512 tokens, transposes to [DIM, tok], AllGather.
- Layer 0 (non-recurrent: h_prev=c_prev=0) + layer-1 x-gates: big parallel matmuls.
- Layer 1 recurrence: per-step [128,128] gate matmuls + 8KB AllGather of h.T.
- Decoder: vocab-sharded (4000/core), interleaved into recurrence PE gaps.
All matmuls bf16 (fp32 PSUM accumulate); elementwise fp32.
"""
import sys

if "/opt/trn_rl_repo" not in sys.path:
    sys.path.insert(0, "/opt/trn_rl_repo")

import numpy as np
import ml_dtypes

BF16 = ml_dtypes.bfloat16
NVOC, DIM, H, L = 32000, 1024, 1024, 2
T, B = 128, 32
TB = T * B              # 4096
R = 8                   # cores
HSL = H // R            # 128  per-core H slice
TSL = TB // R           # 512  per-core token slice
VSL = NVOC // R         # 4000 per-core vocab slice
KT = H // 128           # 8 K tiles
NT = TB // 512          # 8 token N tiles
MT = TB // 128          # 32 decoder M tiles
NVT = 8                 # vocab N tiles per core: 7x512 + 416

_CACHE = {}


def _build():
    import concourse.bass as bass
    import concourse.mybir as mybir
    import concourse.tile as tile
    from concourse import bacc
    from concourse.masks import make_identity

    dt = mybir.dt
    f32, bf16, i32 = dt.float32, dt.bfloat16, dt.int32
    AF = mybir.ActivationFunctionType
    ALU = mybir.AluOpType

    nc = bacc.Bacc("TRN2", target_bir_lowering=False, debug=False, num_devices=R)

    # ---- I/O ----
    enc_w = nc.dram_tensor("enc_w", [NVOC, DIM], f32, kind="ExternalInput").ap()
    tok_idx = nc.dram_tensor("tok_idx", [128, 4], i32, kind="ExternalInput").ap()
    u0t = nc.dram_tensor("u0t", [DIM, 3 * 128], bf16, kind="ExternalInput").ap()
    b0 = nc.dram_tensor("b0", [128, 3], f32, kind="ExternalInput").ap()
    u1t = nc.dram_tensor("u1t", [DIM, 4 * 128], bf16, kind="ExternalInput").ap()
    b1 = nc.dram_tensor("b1", [128, 4], f32, kind="ExternalInput").ap()
    w1t = nc.dram_tensor("w1t", [H, 4 * 128], bf16, kind="ExternalInput").ap()
    decwt = nc.dram_tensor("decwt", [H, VSL], bf16, kind="ExternalInput").ap()
    out_d = nc.dram_tensor("out", [TB, VSL], f32, kind="ExternalOutput").ap()

    # ---- internal DRAM (collective bounce buffers) ----
    embT_dram = nc.dram_tensor("embT_dram", [DIM, TSL], bf16).ap()
    embT_ag = nc.dram_tensor("embT_ag", [R * DIM, TSL], bf16, addr_space="Shared").ap()
    h0T_dram = nc.dram_tensor("h0T_dram", [HSL, TB], bf16).ap()
    h0T_ag = nc.dram_tensor("h0T_ag", [H, TB], bf16, addr_space="Shared").ap()
    hb_d = [nc.dram_tensor(f"hb_{t}", [HSL, B], bf16).ap() for t in range(T)]
    hag_d = [
        nc.dram_tensor(f"hag_{t}", [H, B], bf16, addr_space="Shared").ap()
        for t in range(T)
    ]

    RG = [list(range(R))]
    # views of AG outputs, per token-block n: [p, k, c]
    embT_ag_v = embT_ag.rearrange("(n k p) c -> n p k c", n=R, k=KT)
    h0T_ag_v = h0T_ag.rearrange("(k p) (n c) -> n p k c", k=KT, n=NT)

    with tile.TileContext(nc) as tc:
        with tc.tile_pool(name="glob", bufs=1) as glob:
            w1t_sb = glob.tile([128, KT * 512], bf16)
            nc.sync.dma_start(
                out=w1t_sb[:].rearrange("p (k c) -> p k c", k=KT),
                in_=w1t.rearrange("(k p) c -> p k c", p=128),
            )
            b1_sb = glob.tile([128, 4], f32)
            nc.sync.dma_start(out=b1_sb[:], in_=b1)
            decwt_sb = glob.tile([128, KT * VSL], bf16)  # 62.5KB/part
            nc.sync.dma_start(
                out=decwt_sb[:].rearrange("p (k c) -> p k c", k=KT),
                in_=decwt.rearrange("(k p) c -> p k c", p=128),
            )
            # xg: per step t a [128,128] block: cols [f,i,o,cn] x 32
            xg_sb = glob.tile([128, T * 128], bf16)  # 32KB/part
            xg_view = xg_sb[:].rearrange("p (t gb) -> p t gb", gb=128)

            # ================= Phase A1: gather + transpose + AG =================
            with (
                tc.tile_pool(name="phA", bufs=1) as phA,
                tc.tile_pool(name="phAd", bufs=2) as phAd,
                tc.tile_pool(name="psA", bufs=2, space="PSUM") as psA,
            ):
                ident = phA.tile([128, 128], bf16)
                make_identity(nc, ident[:])
                tok_sb = phA.tile([128, 4], i32)
                nc.sync.dma_start(out=tok_sb[:], in_=tok_idx)
                embbf = phA.tile([128, 4 * DIM], bf16)
                for j in range(4):
                    g32 = phAd.tile([128, DIM], f32, tag="g32")
                    nc.gpsimd.indirect_dma_start(
                        out=g32[:],
                        out_offset=None,
                        in_=enc_w,
                        in_offset=bass.IndirectOffsetOnAxis(
                            ap=tok_sb[:, j : j + 1], axis=0
                        ),
                    )
                    nc.vector.tensor_copy(
                        out=embbf[:, j * DIM : (j + 1) * DIM], in_=g32[:]
                    )
                embT_sb = phA.tile([128, KT * 512], bf16)
                for j in range(4):
                    for k in range(KT):
                        pst = psA.tile([128, 128], bf16, tag="pst")
                        nc.tensor.transpose(
                            pst[:],
                            embbf[:, j * DIM + k * 128 : j * DIM + (k + 1) * 128],
                            ident[:],
                        )
                        nc.scalar.copy(
                            out=embT_sb[:, k * 512 + j * 128 : k * 512 + (j + 1) * 128],
                            in_=pst[:],
                        )
                nc.sync.dma_start(
                    out=embT_dram.rearrange("(k p) c -> p k c", p=128),
                    in_=embT_sb[:].rearrange("p (k c) -> p k c", k=KT),
                )
                nc.gpsimd.collective_compute(
                    "AllGather", ALU.bypass, replica_groups=RG,
                    ins=[embT_dram], outs=[embT_ag],
                )

            # ================= Phase A2: layer 0 =================
            with (
                tc.tile_pool(name="phA2", bufs=1) as phA2,
                tc.tile_pool(name="phA2d", bufs=2) as phA2d,
                tc.tile_pool(name="psB", bufs=4, space="PSUM") as psB,
            ):
                u0t_sb = phA2.tile([128, KT * 384], bf16)
                nc.sync.dma_start(
                    out=u0t_sb[:].rearrange("p (k c) -> p k c", k=KT),
                    in_=u0t.rearrange("(k p) c -> p k c", p=128),
                )
                b0_sb = phA2.tile([128, 3], f32)
                nc.sync.dma_start(out=b0_sb[:], in_=b0)
                h0T_sb = phA2.tile([128, TB], bf16)
                for n in range(NT):
                    embT_n = phA2d.tile([128, KT * 512], bf16, tag="embT_n")
                    nc.sync.dma_start(
                        out=embT_n[:].rearrange("p (k c) -> p k c", k=KT),
                        in_=embT_ag_v[n],
                    )
                    gt = []
                    for g in range(3):
                        ps = psB.tile([128, 512], f32, tag="ps0")
                        for k in range(KT):
                            nc.tensor.matmul(
                                ps[:],
                                lhsT=u0t_sb[:, k * 384 + g * 128 : k * 384 + (g + 1) * 128],
                                rhs=embT_n[:, k * 512 : (k + 1) * 512],
                                start=(k == 0),
                                stop=(k == KT - 1),
                            )
                        act = phA2d.tile([128, 512], f32, tag=f"l0g{g}", name=f"l0g{g}")
                        nc.scalar.activation(
                            out=act[:], in_=ps[:],
                            func=(AF.Sigmoid if g < 2 else AF.Tanh),
                            bias=b0_sb[:, g : g + 1],
                        )
                        gt.append(act)
                    gi, go, gc = gt
                    c0 = phA2d.tile([128, 512], f32, tag="c0")
                    nc.vector.tensor_mul(out=c0[:], in0=gi[:], in1=gc[:])
                    t0 = phA2d.tile([128, 512], f32, tag="t0")
                    nc.scalar.activation(out=t0[:], in_=c0[:], func=AF.Tanh)
                    nc.vector.tensor_mul(
                        out=h0T_sb[:, n * 512 : (n + 1) * 512], in0=go[:], in1=t0[:]
                    )
                nc.sync.dma_start(out=h0T_dram, in_=h0T_sb[:])
                nc.gpsimd.collective_compute(
                    "AllGather", ALU.bypass, replica_groups=RG,
                    ins=[h0T_dram], outs=[h0T_ag],
                )

            # ================= Phase A3: layer-1 x-gates =================
            with (
                tc.tile_pool(name="phA3", bufs=1) as phA3,
                tc.tile_pool(name="phA3d", bufs=2) as phA3d,
                tc.tile_pool(name="psB2", bufs=4, space="PSUM") as psB2,
            ):
                u1t_sb = phA3.tile([128, KT * 512], bf16)
                nc.sync.dma_start(
                    out=u1t_sb[:].rearrange("p (k c) -> p k c", k=KT),
                    in_=u1t.rearrange("(k p) c -> p k c", p=128),
                )
                for n in range(NT):
                    h0T_n = phA3d.tile([128, KT * 512], bf16, tag="h0T_n")
                    nc.sync.dma_start(
                        out=h0T_n[:].rearrange("p (k c) -> p k c", k=KT),
                        in_=h0T_ag_v[n],
                    )
                    for g in range(4):
                        ps = psB2.tile([128, 512], f32, tag="ps1")
                        for k in range(KT):
                            nc.tensor.matmul(
                                ps[:],
                                lhsT=u1t_sb[:, k * 512 + g * 128 : k * 512 + (g + 1) * 128],
                                rhs=h0T_n[:, k * 512 : (k + 1) * 512],
                                start=(k == 0),
                                stop=(k == KT - 1),
                            )
                        # scatter: psum col (32j+b) -> step t=16n+j, col 32g+b
                        nc.scalar.activation(
                            out=xg_view[:, 16 * n : 16 * (n + 1), 32 * g : 32 * (g + 1)],
                            in_=ps[:].rearrange("p (j b) -> p j b", b=32),
                            func=AF.Identity,
                            bias=b1_sb[:, g : g + 1],
                        )

            # ================= Phase B (recurrence) + C (decoder) =================
            with (
                tc.tile_pool(name="pers2", bufs=1) as pers2,
                tc.tile_pool(name="phB", bufs=3) as phB,
                tc.tile_pool(name="psC", bufs=2, space="PSUM") as psC,
                tc.tile_pool(name="psD", bufs=3, space="PSUM") as psD,
                tc.tile_pool(name="phC", bufs=4) as phC,
            ):
                # ysT: gathered h.T per step; col k*TB + t*32 + b
                ysT = pers2.tile([128, KT * TB], bf16)  # 64KB/part
                ysT_k = ysT[:].rearrange("p (k tb) -> p k tb", k=KT)
                c_tiles = [
                    pers2.tile([128, B], f32, tag=f"c{i}", name=f"c{i}")
                    for i in range(2)
                ]

                dec_q = [(m, nv) for m in range(MT) for nv in range(NVT)]
                dec_pos = 0

                def emit_dec_group(m, nv):
                    w0 = nv * 512
                    w1 = min(VSL, w0 + 512)
                    nvw = w1 - w0
                    ps2 = psD.tile([128, 512], f32, tag="psD", name="psD")
                    for k in range(KT):
                        nc.tensor.matmul(
                            ps2[:, :nvw],
                            lhsT=ysT[:, k * TB + m * 128 : k * TB + (m + 1) * 128],
                            rhs=decwt_sb[:, k * VSL + w0 : k * VSL + w1],
                            start=(k == 0),
                            stop=(k == KT - 1),
                        )
                    stage = phC.tile([128, 512], f32, tag="stage", name="stage")
                    nc.scalar.copy(out=stage[:, :nvw], in_=ps2[:, :nvw])
                    nc.sync.dma_start(
                        out=out_d[m * 128 : (m + 1) * 128, w0:w1],
                        in_=stage[:, :nvw],
                    )

                for t in range(T):
                    c_prev = c_tiles[(t + 1) % 2]
                    c_new = c_tiles[t % 2]
                    gsig = phB.tile([128, 96], f32, tag="gsig")
                    gtan = phB.tile([128, 32], f32, tag="gtan")
                    if t == 0:
                        nc.scalar.activation(
                            out=gsig[:], in_=xg_sb[:, 0:96], func=AF.Sigmoid
                        )
                        nc.scalar.activation(
                            out=gtan[:], in_=xg_sb[:, 96:128], func=AF.Tanh
                        )
                        # c = i * cn   (c_prev = 0)
                        nc.vector.tensor_mul(
                            out=c_new[:], in0=gsig[:, 32:64], in1=gtan[:]
                        )
                    else:
                        ps = psC.tile([128, 128], f32, tag="psC")
                        for g in range(4):
                            for k in range(KT):
                                nc.tensor.matmul(
                                    ps[:, 32 * g : 32 * (g + 1)],
                                    lhsT=w1t_sb[:, k * 512 + g * 128 : k * 512 + (g + 1) * 128],
                                    rhs=ysT[:, k * TB + (t - 1) * B : k * TB + t * B],
                                    start=(k == 0),
                                    stop=(k == KT - 1),
                                )
                        pre = phB.tile([128, 128], f32, tag="pre")
                        nc.vector.tensor_tensor(
                            out=pre[:], in0=ps[:],
                            in1=xg_sb[:, t * 128 : (t + 1) * 128], op=ALU.add,
                        )
                        nc.scalar.activation(
                            out=gsig[:], in_=pre[:, 0:96], func=AF.Sigmoid
                        )
                        nc.scalar.activation(
                            out=gtan[:], in_=pre[:, 96:128], func=AF.Tanh
                        )
                        tmpf = phB.tile([128, 32], f32, tag="tmpf")
                        nc.vector.tensor_mul(out=tmpf[:], in0=gsig[:, 0:32], in1=c_prev[:])
                        tmpi = phB.tile([128, 32], f32, tag="tmpi")
                        nc.vector.tensor_mul(out=tmpi[:], in0=gsig[:, 32:64], in1=gtan[:])
                        nc.vector.tensor_add(out=c_new[:], in0=tmpf[:], in1=tmpi[:])
                    tct = phB.tile([128, 32], f32, tag="tct")
                    nc.scalar.activation(out=tct[:], in_=c_new[:], func=AF.Tanh)
                    h_bf = phB.tile([128, B], bf16, tag="h_bf")
                    nc.vector.tensor_mul(out=h_bf[:], in0=gsig[:, 64:96], in1=tct[:])
                    nc.sync.dma_start(out=hb_d[t], in_=h_bf[:])
                    nc.gpsimd.collective_compute(
                        "AllGather", ALU.bypass, replica_groups=RG,
                        ins=[hb_d[t]], outs=[hag_d[t]],
                    )
                    nc.sync.dma_start(
                        out=ysT_k[:, :, t * B : (t + 1) * B],
                        in_=hag_d[t].rearrange("(k p) b -> p k b", p=128),
                    )
                    # interleave decoder groups whose M-tile (4 steps) is complete
                    emitted = 0
                    while (
                        dec_pos < len(dec_q)
                        and emitted < 3
                        and dec_q[dec_pos][0] * 4 + 3 < t
                    ):
                        emit_dec_group(*dec_q[dec_pos])
                        dec_pos += 1
                        emitted += 1
                while dec_pos < len(dec_q):
                    emit_dec_group(*dec_q[dec_pos])
                    dec_pos += 1

    nc.compile()
    return nc


def _get_nc():
    if "nc" not in _CACHE:
        _CACHE["nc"] = _build()
    return _CACHE["nc"]


def _prep_inputs(inputs):
    f = {k: np.asarray(v) for k, v in inputs.items()}
    tokens = f["tokens"].astype(np.int32).reshape(-1)  # [TB] row-major (t, b)
    gates1 = ["f", "i", "o", "c"]  # layer-1 order
    gates0 = ["i", "o", "c"]      # layer-0 order (f unused: c_prev=0)
    in_maps = []
    for r in range(R):
        hs = slice(HSL * r, HSL * (r + 1))
        vs = slice(VSL * r, VSL * (r + 1))
        u0t = np.concatenate(
            [f[f"u{g}_w"][hs, :].T for g in gates0], axis=1
        ).astype(BF16)
        b0 = np.stack(
            [(f[f"u{g}_b"] + f[f"w{g}_b"])[hs] for g in gates0], axis=1
        ).astype(np.float32)
        u1t = np.concatenate(
            [f[f"u{g}_w"][hs, :].T for g in gates1], axis=1
        ).astype(BF16)
        b1 = np.stack(
            [(f[f"u{g}_b"] + f[f"w{g}_b"])[hs] for g in gates1], axis=1
        ).astype(np.float32)
        w1t = np.concatenate(
            [f[f"w{g}_w"][hs, :].T for g in gates1], axis=1
        ).astype(BF16)
        decwt = np.ascontiguousarray(f["dec_w"][vs, :].T).astype(BF16)
        tok_r = tokens[TSL * r : TSL * (r + 1)].reshape(4, 128).T.copy()  # [128,4]
        in_maps.append(
            {
                "enc_w": f["enc_w"].astype(np.float32),
                "tok_idx": np.ascontiguousarray(tok_r),
                "u0t": np.ascontiguousarray(u0t),
                "b0": np.ascontiguousarray(b0),
                "u1t": np.ascontiguousarray(u1t),
                "b1": np.ascontiguousarray(b1),
                "w1t": np.ascontiguousarray(w1t),
                "decwt": decwt,
            }
        )
    dec_b = f["dec_b"].astype(np.float32)
    return in_maps, dec_b


def run_sharded(inputs, trace=False):
    from concourse.bass_utils import run_bass_kernel_spmd

    nc = _get_nc()
    in_maps, dec_b = _prep_inputs(inputs)
    res = run_bass_kernel_spmd(
        nc, in_maps, core_ids=list(range(R)), trace=trace
    )
    parts = [np.asarray(res.results[r]["out"]) for r in range(R)]
    full = np.concatenate(parts, axis=1)  # [TB, NVOC]
    full = full + dec_b[None, :]
    return full.reshape(T, B, NVOC).astype(np.float32), res


def kernel(**inputs):
    out, _ = run_sharded(inputs, trace=False)
    return out
